# revision 10
# baseline (speedup 1.0000x reference)
"""Trainium2 Bass kernel for a noisy LSTMCell forward.

  gates = input @ W_ih.T + b_ih + hx @ W_hh.T + b_hh          # [B, 4H]
  i, f, g, o = split(gates); i,f,o=sigmoid, g=tanh
  cy = f*cx + i*g + sqrt(noise_e)*eps_c
  hy = o*tanh(cy) + sqrt(noise_q)*eps_h

B=4096, I=H=1024. Sharding: 2D grid over 8 NeuronCores — 4 batch shards
x 2 gate shards (25.2MB of matmul inputs per core, the minimum for any
integer grid). Device layout is feature-major ([feature, batch]): the
contraction dim lands on SBUF partitions with zero on-device transposes
and the ACT engine applies the per-partition gate bias during PSUM
eviction. Matmuls run in float32r (full PE rate at free-dim >= 256;
plain bf16 fails the 2e-2 tolerance, measured 6.4e-2).

Schedule (cost model: 129us/exec, PE busy 111us = 86%):
  - host prep: W concat/pre-tiled per block, bias_ih+bias_hh folded,
    sqrt(noise_*) folded into eps; cx/eps shipped bf16; outputs bf16
    (upcast on host) -> 30.4MB HBM traffic per core
  - HWDGE ring = dedicated w/x stream: w0 w1 w2 | x0 x1 x2 | w3 |
    x3..x15 | w4..; elementwise loads + cy stores ride SWDGE; hy stores
    (already bf16) ride HWDGE at the tail when it is idle
  - h-tile 0 runs its 8 accumulation chains (4 gates x 2 batch chunks)
    kt-interleaved across all 8 PSUM banks so every arriving 1MB x tile
    feeds 8 matmuls and the PE tracks the DMA stream densely
  - h-tiles 1-2: kt-major per block (one weight load serves both chunks)
  - last h-tile: chunk-0 chains then its elementwise (overlaps chunk-1
    matmuls); chunk-1 evicts cell/in/forget gates first, interleaves the
    cy-path elementwise between chains, and splits the final out-gate
    chain into two 256-wide halves so only ~3us of evict+mul+add+store
    trail the last matmul
"""

import os
import sys
import numpy as np

for _p in ("/opt/trn_rl_repo", "/root/.axon_site/_ro/trn_rl_repo"):
    if _p not in sys.path and os.path.isdir(_p):
        sys.path.append(_p)

B, I, H = 4096, 1024, 1024
G = 4 * H                 # gate rows total
K = I + H                 # contraction dim
P_B, P_G = 4, 2           # batch shards x gate shards = 8 cores
BS = B // P_B             # 1024 batch cols per core
HS = H // P_G             # 512 h rows per core
NKT = K // 128            # 16 contraction tiles
NHT = HS // 128           # 4 h tiles per core
NA = NHT * 4              # 16 weight blocks (ht-major, gate-minor)
NBC = BS // 512           # 2 batch chunks of 512 (fp32 PSUM free-dim max)

_LAST = None              # BassKernelResults of the most recent run (for test.py)


def _build_nc(mm_dt="float32r", wbufs=6):
    import concourse.bacc as bacc
    import concourse.tile as tile
    from concourse import mybir
    from contextlib import ExitStack

    f32 = mybir.dt.float32
    bf16 = mybir.dt.bfloat16
    mdt = getattr(mybir.dt, mm_dt)
    AF = mybir.ActivationFunctionType
    nc = bacc.Bacc("TRN2", target_bir_lowering=False)

    xT = nc.declare_dram_parameter("xT", [K, BS], mdt, isOutput=False)
    w = nc.declare_dram_parameter("w", [NA, 128, NKT * 128], mdt, isOutput=False)
    bias = nc.declare_dram_parameter("bias", [128, NA], f32, isOutput=False)
    cxT = nc.declare_dram_parameter("cxT", [HS, BS], bf16, isOutput=False)
    epcT = nc.declare_dram_parameter("epcT", [HS, BS], bf16, isOutput=False)
    ephT = nc.declare_dram_parameter("ephT", [HS, BS], bf16, isOutput=False)
    hyT = nc.declare_dram_parameter("hyT", [HS, BS], bf16, isOutput=True)
    cyT = nc.declare_dram_parameter("cyT", [HS, BS], bf16, isOutput=True)

    with tile.TileContext(nc) as tc, ExitStack() as ctx:
        xpool = ctx.enter_context(tc.tile_pool(name="xpool", bufs=1))
        wpool = ctx.enter_context(tc.tile_pool(name="wpool", bufs=wbufs))
        psum = ctx.enter_context(tc.tile_pool(name="psum", bufs=1, space="PSUM"))
        gates = ctx.enter_context(tc.tile_pool(name="gates", bufs=2))
        ew = ctx.enter_context(tc.tile_pool(name="ew", bufs=2))
        const = ctx.enter_context(tc.tile_pool(name="const", bufs=1))

        ps_cnt = [0]

        def ps_tile():
            t = psum.tile([128, 512], f32, tag=f"ps{ps_cnt[0] % 8}",
                          name=f"ps{ps_cnt[0] % 8}")
            ps_cnt[0] += 1
            return t

        bias_t = const.tile([128, NA], f32)
        nc.sync.dma_start(out=bias_t[:], in_=bias[:, :])

        w_tiles = {}

        def load_w(a):
            t = wpool.tile([128, NKT * 128], mdt)
            nc.sync.dma_start(out=t[:], in_=w[a, :, :])
            w_tiles[a] = t

        xk = []

        def load_x(kt):
            t = xpool.tile([128, BS], mdt, tag=f"xk{kt}", name=f"xk{kt}")
            nc.sync.dma_start(out=t[:], in_=xT[kt * 128:(kt + 1) * 128, :])
            xk.append(t)

        # head wire: w0 w1 w2 | x0 x1 x2 | w3 | x3..x15 — six accumulation
        # chains track the first x tiles, all eight once w3 lands
        load_w(0), load_w(1), load_w(2)
        load_x(0), load_x(1), load_x(2)
        load_w(3)
        for kt in range(3, NKT):
            load_x(kt)

        dma = nc.gpsimd

        def emit_ew(ht, gt, chunk=None):
            i_t, f_t, g_t, o_t = gt
            row = slice(ht * 128, (ht + 1) * 128)
            if chunk is None:
                cols, n = slice(0, BS), BS
            else:
                cols, n = slice(chunk * 512, (chunk + 1) * 512), 512
            cx_t = ew.tile([128, n], bf16, tag=f"cx{chunk}")
            ec_t = ew.tile([128, n], bf16, tag=f"ec{chunk}")
            eh_t = ew.tile([128, n], bf16, tag=f"eh{chunk}")
            dma.dma_start(out=cx_t[:], in_=cxT[row, cols])
            dma.dma_start(out=ec_t[:], in_=epcT[row, cols])
            dma.dma_start(out=eh_t[:], in_=ephT[row, cols])
            hy_t = ew.tile([128, n], bf16, tag=f"hyo{chunk}")
            fv, iv, gv, ov = f_t[:, cols], i_t[:, cols], g_t[:, cols], o_t[:, cols]
            nc.vector.tensor_mul(fv, fv, cx_t[:])
            nc.vector.tensor_mul(iv, iv, gv)
            nc.vector.tensor_add(fv, fv, iv)
            nc.vector.tensor_add(fv, fv, ec_t[:])
            nc.scalar.activation(gv, fv, AF.Tanh)
            nc.vector.tensor_mul(ov, ov, gv)
            nc.vector.tensor_add(hy_t[:], ov, eh_t[:])
            nc.gpsimd.dma_start(out=cyT[row, cols], in_=fv)   # cast fp32->bf16
            nc.sync.dma_start(out=hyT[row, cols], in_=hy_t[:])

        for ht in range(NHT):
            gt = [gates.tile([128, BS], f32, tag=f"g{g}", name=f"gt{g}")
                  for g in range(4)]

            def evict(gate, c, ps):
                a = ht * 4 + gate
                func = AF.Tanh if gate == 2 else AF.Sigmoid
                nc.scalar.activation(gt[gate][:, c * 512:(c + 1) * 512],
                                     ps[:], func, bias=bias_t[:, a:a + 1])

            def chain(gate, c, w_t, ps):
                for kt in range(NKT):
                    nc.tensor.matmul(
                        ps[:], w_t[:, kt * 128:(kt + 1) * 128],
                        xk[kt][:, c * 512:(c + 1) * 512],
                        start=(kt == 0), stop=(kt == NKT - 1))
                evict(gate, c, ps)

            if ht == 0:
                # kt-interleaved: 8 open chains across all 8 PSUM banks
                wts = {}
                for gate in range(4):
                    wts[gate] = w_tiles.pop(gate)
                pss = {(g, c): ps_tile() for g in range(4) for c in range(NBC)}
                for kt in range(NKT):
                    for g in range(4):
                        for c in range(NBC):
                            nc.tensor.matmul(
                                pss[(g, c)][:],
                                wts[g][:, kt * 128:(kt + 1) * 128],
                                xk[kt][:, c * 512:(c + 1) * 512],
                                start=(kt == 0), stop=(kt == NKT - 1))
                            if kt == NKT - 1:
                                evict(g, c, pss[(g, c)])
                emit_ew(ht, gt)
            elif ht == NHT - 1:
                wts = {}
                for gate in range(4):
                    a = ht * 4 + gate
                    if a not in w_tiles:
                        load_w(a)
                    wts[gate] = w_tiles.pop(a)
                for gate in range(4):
                    chain(gate, 0, wts[gate], ps_tile())
                emit_ew(ht, gt, chunk=0)
                row = slice(ht * 128, (ht + 1) * 128)
                cols = slice(512, 1024)
                i_t, f_t, g_t, o_t = gt
                cx_t = ew.tile([128, 512], bf16, tag="cx1")
                ec_t = ew.tile([128, 512], bf16, tag="ec1")
                eh_t = ew.tile([128, 512], bf16, tag="eh1")
                nc.gpsimd.dma_start(out=cx_t[:], in_=cxT[row, cols])
                nc.gpsimd.dma_start(out=ec_t[:], in_=epcT[row, cols])
                nc.gpsimd.dma_start(out=eh_t[:], in_=ephT[row, cols])
                hy_t = ew.tile([128, 512], bf16, tag="hyo1")
                fv, iv = f_t[:, cols], i_t[:, cols]
                gv, ov = g_t[:, cols], o_t[:, cols]
                chain(2, 1, wts[2], ps_tile())          # cellgate
                chain(0, 1, wts[0], ps_tile())          # ingate
                nc.vector.tensor_mul(iv, iv, gv)        # i*g
                chain(1, 1, wts[1], ps_tile())          # forgetgate
                nc.vector.tensor_mul(fv, fv, cx_t[:])   # f*cx
                nc.vector.tensor_add(fv, fv, iv)
                nc.vector.tensor_add(fv, fv, ec_t[:])   # = cy
                nc.scalar.activation(gv, fv, AF.Tanh)   # tanh(cy)
                nc.gpsimd.dma_start(out=cyT[row, cols], in_=fv)
                # outgate last, split into two 256-wide chains: the first
                # half's evict/elementwise/store overlap the second half's
                # matmuls, so only ~256 cols of tail trail the final MM
                for h in range(2):
                    hc = slice(512 + h * 256, 512 + (h + 1) * 256)
                    lc = slice(h * 256, (h + 1) * 256)
                    ps = ps_tile()
                    for kt in range(NKT):
                        nc.tensor.matmul(
                            ps[:, 0:256], wts[3][:, kt * 128:(kt + 1) * 128],
                            xk[kt][:, hc], start=(kt == 0), stop=(kt == NKT - 1))
                    a = ht * 4 + 3
                    nc.scalar.activation(o_t[:, hc], ps[:, 0:256], AF.Sigmoid,
                                         bias=bias_t[:, a:a + 1])
                    nc.vector.tensor_mul(o_t[:, hc], o_t[:, hc], g_t[:, hc])
                    nc.vector.tensor_add(hy_t[:, lc], o_t[:, hc], eh_t[:, lc])
                    nc.sync.dma_start(out=hyT[row, hc], in_=hy_t[:, lc])
            else:
                for gate in range(4):
                    a = ht * 4 + gate
                    if a not in w_tiles:
                        load_w(a)
                    w_t = w_tiles.pop(a)
                    ps = [ps_tile() for _ in range(NBC)]
                    for c in range(NBC):
                        chain(gate, c, w_t, ps[c])
                emit_ew(ht, gt)

    nc.compile()
    return nc


def _prep_inputs(input, hx, cx, noise_q, noise_e,
                 weight_ih, weight_hh, bias_ih, bias_hh, eps_c, eps_h,
                 ew_bf16=True):
    import ml_dtypes
    ewdt = np.dtype(ml_dtypes.bfloat16) if ew_bf16 else np.float32
    f = lambda a: np.ascontiguousarray(np.asarray(a, dtype=np.float32))
    X = np.concatenate([f(input), f(hx)], axis=1)          # [B, K]
    XT = np.ascontiguousarray(X.T)                          # [K, B]
    W_cat = np.concatenate([f(weight_ih), f(weight_hh)], axis=1)   # [G, K]
    bias_full = f(bias_ih) + f(bias_hh)                     # [G]
    se = np.sqrt(np.float32(np.asarray(noise_e).reshape(-1)[0]))
    sq = np.sqrt(np.float32(np.asarray(noise_q).reshape(-1)[0]))
    cxT = f(cx).T.astype(ewdt)
    epcT = (f(eps_c) * se).T.astype(ewdt)   # sqrt(noise) folded on host
    ephT = (f(eps_h) * sq).T.astype(ewdt)

    # Per gate-shard j: weight blocks in consumption order (a = ht*4+gate),
    # pre-transposed to [k_p, kt*128 + g_c] so each DMA partition line is one
    # contiguous 8KB run.
    w_host, bias_host = [], []
    for j in range(P_G):
        blocks, bcols = [], []
        for ht in range(NHT):
            for gate in range(4):
                g0 = gate * H + j * HS + ht * 128
                blk = W_cat[g0:g0 + 128, :]                        # (g_c, k)
                blocks.append(blk.reshape(128, NKT, 128).transpose(2, 1, 0))
                bcols.append(bias_full[g0:g0 + 128])
        w_host.append(np.ascontiguousarray(
            np.stack(blocks).reshape(NA, 128, NKT * 128)))
        bias_host.append(np.ascontiguousarray(np.stack(bcols, axis=1)))

    in_maps = []
    for bi in range(P_B):
        bcol = slice(bi * BS, (bi + 1) * BS)
        for j in range(P_G):
            hrow = slice(j * HS, (j + 1) * HS)
            in_maps.append({
                "xT": np.ascontiguousarray(XT[:, bcol]),
                "w": w_host[j],
                "bias": bias_host[j],
                "cxT": np.ascontiguousarray(cxT[hrow, bcol]),
                "epcT": np.ascontiguousarray(epcT[hrow, bcol]),
                "ephT": np.ascontiguousarray(ephT[hrow, bcol]),
            })
    return in_maps


def _gather(results):
    hyT = np.empty((H, B), dtype=np.float32)
    cyT = np.empty((H, B), dtype=np.float32)
    idx = 0
    for bi in range(P_B):
        bcol = slice(bi * BS, (bi + 1) * BS)
        for j in range(P_G):
            hrow = slice(j * HS, (j + 1) * HS)
            hyT[hrow, bcol] = np.asarray(results[idx]["hyT"]).astype(np.float32)
            cyT[hrow, bcol] = np.asarray(results[idx]["cyT"]).astype(np.float32)
            idx += 1
    return np.ascontiguousarray(hyT.T), np.ascontiguousarray(cyT.T)


def kernel(**inputs):
    global _LAST
    from concourse.bass_utils import run_bass_kernel_spmd

    in_maps = _prep_inputs(**inputs)
    nc = _build_nc()
    _LAST = run_bass_kernel_spmd(nc, in_maps, list(range(8)), trace=False)
    return _gather(_LAST.results)


# ---------------------------------------------------------------------------
# Timing helper for test.py (not used by the grading path): pipelined async
# dispatches with device-resident inputs; the (t_N - t_1)/(N-1) slope is the
# per-execution time as seen by the dispatch stream.
# ---------------------------------------------------------------------------

def benchmark(inputs, n_iter=41, reps=7):
    in_maps = _prep_inputs(**inputs)
    nc = _build_nc()
    per_exec_ns, t1_ns, results = _bench_nc(nc, in_maps, n_iter, reps)
    return per_exec_ns, t1_ns, _gather(results)


def _bench_nc(nc, in_maps, n_iter=41, reps=7):
    import time
    import jax
    from jax.sharding import Mesh, PartitionSpec, NamedSharding
    from jax.experimental.shard_map import shard_map
    from concourse import bass2jax, mybir
    from concourse.bass2jax import _bass_exec_p

    bass2jax.install_neuronx_cc_hook()
    assert nc.dbg_addr is None
    partition_name = nc.partition_id_tensor.name if nc.partition_id_tensor else None

    in_names, out_names, out_avals, zero_outs = [], [], [], []
    for alloc in nc.m.functions[0].allocations:
        if not isinstance(alloc, mybir.MemoryLocationSet):
            continue
        name = alloc.memorylocations[0].name
        if alloc.kind == "ExternalInput":
            if name != partition_name:
                in_names.append(name)
        elif alloc.kind == "ExternalOutput":
            shape = tuple(alloc.tensor_shape)
            dtype = mybir.dt.np(alloc.dtype)
            out_names.append(name)
            out_avals.append(jax.core.ShapedArray(shape, dtype))
            zero_outs.append(np.zeros(shape, dtype))
    n_params = len(in_names)
    all_in_names = tuple(in_names + out_names
                         + ([partition_name] if partition_name else []))

    def _body(*args):
        ins = list(args[:n_params])
        outs = tuple(args[n_params:])
        pid = [bass2jax.partition_id_tensor()] if partition_name else []
        return tuple(_bass_exec_p.bind(
            *ins, *outs, *pid,
            out_avals=tuple(out_avals),
            in_names=all_in_names,
            out_names=tuple(out_names),
            lowering_input_output_aliases=(),
            sim_require_finite=True,
            sim_require_nnan=True,
            nc=nc,
        ))

    n_cores = 8
    devices = jax.devices()[:n_cores]
    mesh = Mesh(np.asarray(devices), ("core",))
    spec = NamedSharding(mesh, PartitionSpec("core"))
    in_specs = (PartitionSpec("core"),) * (n_params + len(out_names))
    out_specs = (PartitionSpec("core"),) * len(out_names)

    concat_in = [
        np.concatenate([np.asarray(in_maps[c][name]) for c in range(n_cores)], axis=0)
        for name in in_names
    ]
    concat_zeros = [
        np.zeros((n_cores * z.shape[0], *z.shape[1:]), z.dtype) for z in zero_outs
    ]
    dev_args = [jax.device_put(a, spec) for a in concat_in + concat_zeros]
    jax.block_until_ready(dev_args)

    fn = jax.jit(shard_map(_body, mesh=mesh, in_specs=in_specs,
                           out_specs=out_specs, check_rep=False),
                 keep_unused=True)
    out1 = fn(*dev_args)          # compile + warm
    jax.block_until_ready(out1)

    def timed(iters):
        best = float("inf")
        for _ in range(reps):
            t0 = time.perf_counter()
            out = None
            for _i in range(iters):
                out = fn(*dev_args)   # async dispatches queue in order
            jax.block_until_ready(out)
            best = min(best, time.perf_counter() - t0)
        return best

    # Congestion on the axon tunnel is one-sided and bursty: take the best
    # slope over several interleaved rounds.
    per_exec_ns = float("inf")
    t1 = float("inf")
    for _ in range(3):
        a = timed(1)
        b = timed(n_iter)
        t1 = min(t1, a)
        per_exec_ns = min(per_exec_ns, (b - a) / (n_iter - 1) * 1e9)
    results = [
        {name: np.asarray(out1[i]).reshape(n_cores, *out_avals[i].shape)[c]
         for i, name in enumerate(out_names)}
        for c in range(n_cores)
    ]
    return per_exec_ns, t1 * 1e9, results
